# revision 5
# baseline (speedup 1.0000x reference)
"""AdvancedNeuroplasticityLayer — Trainium2 Bass kernel (8-core SPMD).

Reference math (B=128, I=2048, O=2048, SEG=10, all fp32):
    astro_mod = sigmoid(astrocyte_activation * context)            # [O]
    dend      = sum_j relu(einsum('bi,oij->boj', x, DS))           # [B, O]
    out       = x @ (weight * astro_mod[:,None]).T + bias + dend   # [B, O]

Distribution: tensor-parallel shard of the output dim O across the 8
NeuronCores (O_SH = 256 per core).  Each core reads the replicated x
(0.5 MB fp16) plus its own shard of dendrite_segments (10.5 MB fp16) and
of the astro-gated weight (1 MB fp16), computes its [B, 256] output
slice, and the host concatenates the slices ("all-gather" on host).

Per-core kernel structure (DMA-bound at ~360 GB/s/core):
  - Host folds astro_mod into the weight columns and packs x^T / W^T /
    DS^T into [K-partition, column]-major images so every device DMA is a
    dense >=1 MB transfer at line rate.
  - One fused matmul sweep: 2560 dendrite columns (o-major, col =
    o_local*SEG + j) in 5 groups of 512 (one fp32 PSUM bank each), fp16
    operands with fp32 PSUM accumulation over 16 K-tiles; x^T k-tiles are
    the stationary operand.
  - Per group: ScalarE relu PSUM->SBUF; VectorE reduce over SEG on a
    [128, o, 10] strided view, chunked in 64-o pieces gated on the group
    that completes them, so reductions hide under the DMA stream.
  - The gated-linear weight streams last (shortest tail): two 512 KB
    halves, each followed by its 8 accumulating matmuls into its own PSUM
    bank, then osum += psw_h, single [128, 256] fp32 store.

Numerics: fp16 mantissa (11 bits) on the streamed operands, fp32
accumulation everywhere else -> absmax-relative error ~3e-4 vs the fp32
reference (measured 2.85e-4 on the reference inputs; modeled 42 us/core).
Exact-fp32 fallback: set MM_DT = F32 / NP_DT = np.float32 below (rel err
7e-7, modeled 111 us/core — fp32 matmuls run at 1/4 rate and the DS
stream doubles).
"""

import numpy as np

import concourse.bass as bass  # noqa: F401  (bass types referenced via bacc)
import concourse.tile as tile
from concourse import bacc, mybir
from concourse import bass_utils

B, I, O, SEG = 128, 2048, 2048, 10
NCORES = 8
O_SH = O // NCORES            # 256 output columns per core
KT = I // 128                 # 16 contraction tiles
KH = KT // 2                  # 8 k-tiles per half-group DMA
DCOLS = O_SH * SEG            # 2560 dendrite columns per core
NG = 512                      # group width = one fp32 PSUM bank
GROUPS = DCOLS // NG          # 5

F16 = mybir.dt.float16
F32 = mybir.dt.float32

# Streamed-operand precision knob (see module docstring).
MM_DT = F16
NP_DT = np.float16


def build_nc(mm_dt=MM_DT, ds_bufs=4):
    nc = bacc.Bacc("TRN2", target_bir_lowering=False, debug=False)

    xT = nc.dram_tensor("xT", [128, KT * B], mm_dt, kind="ExternalInput").ap()
    ds = nc.dram_tensor(
        "ds", [GROUPS, 2, 128, KH * NG], mm_dt, kind="ExternalInput"
    ).ap()
    wg = nc.dram_tensor("wg", [128, KT * O_SH], mm_dt, kind="ExternalInput").ap()
    bb = nc.dram_tensor("biasb", [B, O_SH], F32, kind="ExternalInput").ap()
    out = nc.dram_tensor("out", [B, O_SH], F32, kind="ExternalOutput").ap()

    with tile.TileContext(nc) as tc:
        with (
            tc.tile_pool(name="xw", bufs=1) as xwpool,
            tc.tile_pool(name="dst", bufs=ds_bufs) as dspool,
            tc.tile_pool(name="dr", bufs=1) as drpool,
            tc.tile_pool(name="fin", bufs=1) as finpool,
            tc.tile_pool(name="psw", bufs=2, space="PSUM") as pswpool,
            tc.tile_pool(name="psd", bufs=3, space="PSUM") as psdpool,
        ):
            # x^T: one DMA, direct SBUF image [128, 16*128] fp16 (4 KB rows)
            xt_flat = xwpool.tile([128, KT * B], mm_dt)
            nc.sync.dma_start(xt_flat[:], xT[:])
            xt = xt_flat[:].rearrange("p (k m) -> p k m", k=KT)

            wgt_flat = xwpool.tile([128, KT * O_SH], mm_dt)
            bt = finpool.tile([B, O_SH], F32)

            dr = drpool.tile([128, GROUPS, NG], F32)
            dend = finpool.tile([128, O_SH], F32)
            osum = finpool.tile([128, O_SH], F32)
            # W-group accumulates k 0..7 / k 8..15 into separate banks so
            # each half pipelines behind its own 512 KB wgt DMA at the tail.
            psw = [
                pswpool.tile([128, O_SH], F32, name=f"psw{h}") for h in range(2)
            ]
            wgt = wgt_flat[:].rearrange("p (k n) -> p k n", k=KT)

            drv = (
                dr[:]
                .rearrange("p g n -> p (g n)")
                .rearrange("p (o j) -> p o j", j=SEG)
            )  # [128, 256, 10]

            OCH = 64  # o-granularity of the segment reduction

            def reduce_chunk(c):
                nc.vector.reduce_sum(
                    dend[:, c * OCH : (c + 1) * OCH],
                    drv[:, c * OCH : (c + 1) * OCH, :],
                    axis=mybir.AxisListType.X,
                )

            def w_matmuls(h):
                for k in range(KH):
                    nc.tensor.matmul(
                        psw[h][:], xt[:, h * KH + k, :], wgt[:, h * KH + k, :],
                        start=(k == 0), stop=(k == KH - 1),
                    )

            for g in range(GROUPS):
                ps = psdpool.tile([128, NG], F32)
                for h in range(2):
                    dsg = dspool.tile([128, KH * NG], mm_dt)
                    nc.sync.dma_start(dsg[:], ds[g, h])
                    dsgv = dsg[:].rearrange("p (k n) -> p k n", k=KH)
                    for k in range(KH):
                        kk = h * KH + k
                        nc.tensor.matmul(
                            ps[:], xt[:, kk, :], dsgv[:, k, :],
                            start=(kk == 0), stop=(kk == KT - 1),
                        )
                nc.scalar.activation(
                    dr[:, g, :], ps[:], mybir.ActivationFunctionType.Relu
                )
                if g == 0:
                    # bias rides the ACT HWDGE ring; input stream stays dense
                    nc.scalar.dma_start(bt[:], bb[:])
                if g >= 1:
                    reduce_chunk(g - 1)       # chunk c ready after group c+1
                if g == 2:
                    nc.vector.tensor_add(
                        osum[:, 0:128], dend[:, 0:128], bt[:, 0:128]
                    )
            nc.vector.tensor_add(
                osum[:, 128:256], dend[:, 128:256], bt[:, 128:256]
            )

            # gated-linear weight streams last: shortest tail chain
            for h in range(2):
                nc.sync.dma_start(
                    wgt_flat[:, h * KH * O_SH : (h + 1) * KH * O_SH],
                    wg[:, h * KH * O_SH : (h + 1) * KH * O_SH],
                )
                w_matmuls(h)
                nc.vector.tensor_add(osum[:], osum[:], psw[h][:])
            nc.sync.dma_start(out[:], osum[:])

    nc.compile()
    return nc


def prep_inputs(x, context, prev_activation, weight, bias, astrocyte_activation,
                dendrite_segments, np_dt=NP_DT):
    """Host-side shard + pack into the DMA-friendly per-core layouts."""
    x = np.asarray(x, dtype=np.float32)
    weight = np.asarray(weight, dtype=np.float32)
    bias = np.asarray(bias, dtype=np.float32)
    context = np.asarray(context, dtype=np.float32)
    astro = np.asarray(astrocyte_activation, dtype=np.float32)
    ds_full = np.asarray(dendrite_segments, dtype=np.float32)

    astro_mod = 1.0 / (1.0 + np.exp(-(astro * context)))
    wg_full = (weight * astro_mod[:, None]).T.astype(np_dt)       # [I, O]
    wg_k = wg_full.reshape(KT, 128, O)

    # SBUF image: xT_pack[p, k*B+m] = x[m, k*128+p]
    xT_pack = np.ascontiguousarray(
        x.reshape(B, KT, 128).transpose(2, 1, 0).reshape(128, KT * B)
    ).astype(np_dt)

    dsT = ds_full.transpose(1, 0, 2)                              # [I, O, SEG] view

    in_maps = []
    for c in range(NCORES):
        sl = slice(c * O_SH, (c + 1) * O_SH)
        blk = dsT[:, sl, :].astype(np_dt)                         # [I, 256, 10]
        # [G, 2, 128, KH*NG]: half-group h holds k-tiles h*KH..h*KH+KH-1
        ds_pack = np.ascontiguousarray(
            blk.reshape(2, KH, 128, GROUPS, NG).transpose(3, 0, 2, 1, 4)
        ).reshape(GROUPS, 2, 128, KH * NG)
        # SBUF image: wg_pack[p, k*O_SH+n] = wg_k[k, p, sl][n]
        wg_pack = np.ascontiguousarray(
            wg_k[:, :, sl].transpose(1, 0, 2).reshape(128, KT * O_SH)
        )
        bias_b = np.ascontiguousarray(
            np.broadcast_to(bias[sl], (B, O_SH)).astype(np.float32)
        )
        in_maps.append(
            {"xT": xT_pack, "ds": ds_pack, "wg": wg_pack, "biasb": bias_b}
        )
    return in_maps


_NC_CACHE = {}


def get_nc():
    if "nc" not in _NC_CACHE:
        _NC_CACHE["nc"] = build_nc()
    return _NC_CACHE["nc"]


def kernel(**inputs):
    nc = get_nc()
    in_maps = prep_inputs(**inputs)
    res = bass_utils.run_bass_kernel_spmd(
        nc, in_maps, core_ids=list(range(NCORES))
    )
    return np.concatenate(
        [res.results[c]["out"] for c in range(NCORES)], axis=1
    )
